# revision 1
# baseline (speedup 1.0000x reference)
"""Trainium2 Bass kernel for nn_MultiHeadedAttention_33835752358170.

Shapes (hardcoded): x [4, 2048, 1024] f32, w_in [192, 1024], b_in [192],
w_out [1024, 64], b_out [1024].  Module quirk: d_k = 64 total across 16
heads -> head_dim = 4.  Scale is 1/sqrt(64) = 1/8, folded into the q
projection weights on the host.

Sharding: 8 cores = 4 batches x 2 query-halves.  Each core computes
K/V over its batch's full sequence (S=2048) and attention + output
projection for its own 1024 query rows.

Per-core kernel layout choices:
- scores computed TRANSPOSED: S^T[l, sq] tiles [128, 1024] in PSUM via
  K=4 matmuls (per-head).  q^T/k^T live in "strip" layout: head h=4j+s
  occupies partitions [32s, 32s+4) of strip-tile j (satisfies the
  tile_position base-partition rule and enables PE row-group overlap).
- exp on ScalarE reads PSUM scores, writes bf16 SBUF (the only
  transcendental engine; this is the bottleneck ~250us).
- A@V via col-tiled matmuls: head h=4j+s has a private 32-wide lhsT
  window in v_aug (v dims at cols 8s..8s+3, ones col at 8s+4, rest 0);
  out accumulates at PSUM partitions 32j+8s+d over all 16 l-chunks.
  The ones column yields softmax denominators for free.
- normalization via 0/1 select/replicate matmuls + DVE reciprocal,
  then final projection with a slot-permuted w_out (host-built).
"""

import math

import numpy as np
import ml_dtypes

import concourse.bass as bass
import concourse.mybir as mybir
import concourse.tile as tile
from concourse import bacc
from concourse.bass_utils import run_bass_kernel_spmd

BF16 = ml_dtypes.bfloat16
F32 = np.float32

B, S, DM = 4, 2048, 1024
NH, DK = 16, 64
HD = 4          # head dim
SQ = 1024       # query rows per core
NC_CORES = 8

_cache = {}


def _slot(h):
    # head h = 4j+s -> output partition base 32j + 8s (+d, denom at +4)
    j, s = divmod(h, 4)
    return 32 * j + 8 * s


def _build_nc():
    f32 = mybir.dt.float32
    bf16 = mybir.dt.bfloat16
    Exp = mybir.ActivationFunctionType.Exp

    nc = bacc.Bacc("TRN2", target_bir_lowering=False, debug=False)

    # ---- DRAM I/O ----
    d_xT = nc.dram_tensor("xT", [DM, S], bf16, kind="ExternalInput").ap()
    d_xqT = nc.dram_tensor("xqT", [DM, SQ], bf16, kind="ExternalInput").ap()
    d_wq = nc.dram_tensor("wq", [DM, 4, 128], bf16, kind="ExternalInput").ap()
    d_wk = nc.dram_tensor("wk", [DM, 4, 128], bf16, kind="ExternalInput").ap()
    d_wv = nc.dram_tensor("wv", [DM, DK], bf16, kind="ExternalInput").ap()
    d_bq = nc.dram_tensor("bq", [128, 4], f32, kind="ExternalInput").ap()
    d_bk = nc.dram_tensor("bk", [128, 4], f32, kind="ExternalInput").ap()
    d_sel = nc.dram_tensor("sel", [128, NH], f32, kind="ExternalInput").ap()
    d_rep = nc.dram_tensor("rep", [NH, 128], f32, kind="ExternalInput").ap()
    d_wo = nc.dram_tensor("wo", [128, DM], bf16, kind="ExternalInput").ap()
    d_be = nc.dram_tensor("be", [1, DM], f32, kind="ExternalInput").ap()
    d_y = nc.dram_tensor("y", [SQ, DM], f32, kind="ExternalOutput").ap()

    with tile.TileContext(nc) as tc:
        with tc.tile_pool(name="const", bufs=1) as cp:
            # ---- load inputs to SBUF ----
            xT_sb = cp.tile([128, 8, S], bf16)
            xqT_sb = cp.tile([128, 8, SQ], bf16)
            wq_sb = cp.tile([128, 8, 4, 128], bf16)
            wk_sb = cp.tile([128, 8, 4, 128], bf16)
            wv_sb = cp.tile([128, 8, DK], bf16)
            for kc in range(8):
                r = slice(kc * 128, (kc + 1) * 128)
                nc.sync.dma_start(out=xT_sb[:, kc, :], in_=d_xT[r, :])
                nc.sync.dma_start(out=xqT_sb[:, kc, :], in_=d_xqT[r, :])
                nc.sync.dma_start(out=wq_sb[:, kc, :, :], in_=d_wq[r, :, :])
                nc.sync.dma_start(out=wk_sb[:, kc, :, :], in_=d_wk[r, :, :])
                nc.sync.dma_start(out=wv_sb[:, kc, :], in_=d_wv[r, :])
            bq_sb = cp.tile([128, 4], f32)
            bk_sb = cp.tile([128, 4], f32)
            sel_sb = cp.tile([128, NH], f32)
            rep_sb = cp.tile([NH, 128], f32)
            wo_sb = cp.tile([128, DM], bf16)
            be_sb = cp.tile([128, DM], f32)
            nc.sync.dma_start(out=bq_sb, in_=d_bq)
            nc.sync.dma_start(out=bk_sb, in_=d_bk)
            nc.sync.dma_start(out=sel_sb, in_=d_sel)
            nc.sync.dma_start(out=rep_sb, in_=d_rep)
            nc.sync.dma_start(out=wo_sb, in_=d_wo)
            be_b = bass.AP(tensor=d_be.tensor, offset=d_be.offset,
                           ap=[[0, 128], [1, DM]])
            nc.sync.dma_start(out=be_sb, in_=be_b)

            qT = cp.tile([128, 4, SQ], bf16)     # strip g: heads 4g..4g+3
            kT = cp.tile([128, 4, S], bf16)
            v_aug = cp.tile([128, 16, 512], bf16)  # per l-chunk, per head 32w
            outT_sb = cp.tile([128, SQ], f32)

            nc.vector.memset(v_aug, 0.0)
            va4 = v_aug.rearrange("p c (j q) -> p c j q", j=4)
            for s in range(4):
                nc.vector.memset(va4[:, :, :, 40 * s + 4:40 * s + 5], 1.0)

            # ---- projections ----
            with tc.tile_pool(name="pp", bufs=2, space="PSUM") as pp, \
                 tc.tile_pool(name="pv", bufs=2, space="PSUM") as pvp:
                for g in range(4):
                    pt = pp.tile([128, SQ], f32)
                    for nh in range(2):
                        for kc in range(8):
                            nc.tensor.matmul(
                                pt[:, nh * 512:(nh + 1) * 512],
                                wq_sb[:, kc, g, :],
                                xqT_sb[:, kc, nh * 512:(nh + 1) * 512],
                                start=(kc == 0), stop=(kc == 7))
                    nc.vector.tensor_scalar_add(qT[:, g, :], pt, bq_sb[:, g:g + 1])
                for g in range(4):
                    for sh in range(2):
                        pt = pp.tile([128, 1024], f32)
                        for nh in range(2):
                            for kc in range(8):
                                nc.tensor.matmul(
                                    pt[:, nh * 512:(nh + 1) * 512],
                                    wk_sb[:, kc, g, :],
                                    xT_sb[:, kc, sh * 1024 + nh * 512: sh * 1024 + (nh + 1) * 512],
                                    start=(kc == 0), stop=(kc == 7))
                        nc.vector.tensor_scalar_add(
                            kT[:, g, sh * 1024:(sh + 1) * 1024], pt, bk_sb[:, g:g + 1])
                for c in range(16):
                    pv = pvp.tile([128, DK], f32)
                    for kc in range(8):
                        nc.tensor.matmul(
                            pv, xT_sb[:, kc, c * 128:(c + 1) * 128],
                            wv_sb[:, kc, :], start=(kc == 0), stop=(kc == 7))
                    pvr = pv.rearrange("p (j r) -> p j r", j=4)
                    for s in range(4):
                        nc.vector.tensor_copy(
                            va4[:, c, :, 40 * s:40 * s + 4],
                            pvr[:, :, 4 * s:4 * s + 4])

            # ---- attention main loop ----
            with tc.tile_pool(name="op", bufs=1, space="PSUM") as op, \
                 tc.tile_pool(name="sp", bufs=3, space="PSUM") as sp, \
                 tc.tile_pool(name="ep", bufs=6) as ep:
                oT = op.tile([128, SQ], f32)
                for j in range(4):
                    for c in range(16):
                        sts, ets = [], []
                        for s in range(4):
                            st = sp.tile([128, 1024], f32, tag="st")
                            et = ep.tile([128, 1024], bf16, tag="et")
                            sts.append(st)
                            ets.append(et)
                            for nh in range(2):
                                nc.tensor.matmul(
                                    st[:, nh * 512:(nh + 1) * 512],
                                    kT[32 * s:32 * s + 4, j, c * 128:(c + 1) * 128],
                                    qT[32 * s:32 * s + 4, j, nh * 512:(nh + 1) * 512],
                                    start=True, stop=True,
                                    tile_position=(32 * s, 0))
                        for s in range(4):
                            nc.scalar.activation(ets[s], sts[s], Exp)
                        for s in range(4):
                            h = 4 * j + s
                            for nh in range(2):
                                nc.tensor.matmul(
                                    oT[32 * j:32 * j + 32, nh * 512:(nh + 1) * 512],
                                    v_aug[:, c, 32 * h:32 * h + 32],
                                    ets[s][:, nh * 512:(nh + 1) * 512],
                                    start=(c == 0 and s == 0),
                                    stop=(c == 15 and s == 3),
                                    tile_position=(0, 32 * j))
                    nc.vector.tensor_copy(outT_sb[32 * j:32 * j + 32, :],
                                          oT[32 * j:32 * j + 32, :])

            # ---- normalize + output projection ----
            with tc.tile_pool(name="fp", bufs=2, space="PSUM") as fp, \
                 tc.tile_pool(name="np_", bufs=1, space="PSUM") as npp, \
                 tc.tile_pool(name="fs", bufs=2) as fs:
                dn = npp.tile([NH, SQ], f32)
                for nh in range(2):
                    nc.tensor.matmul(dn[:, nh * 512:(nh + 1) * 512], sel_sb,
                                     outT_sb[:, nh * 512:(nh + 1) * 512],
                                     start=True, stop=True)
                rc = cp.tile([NH, SQ], f32)
                nc.vector.reciprocal(rc, dn)
                rp = npp.tile([128, SQ], f32)
                for nh in range(2):
                    nc.tensor.matmul(rp[:, nh * 512:(nh + 1) * 512], rep_sb,
                                     rc[:, nh * 512:(nh + 1) * 512],
                                     start=True, stop=True)
                nrm = cp.tile([128, SQ], bf16)
                nc.vector.tensor_mul(nrm, outT_sb, rp)
                for m in range(8):
                    pf = fp.tile([128, DM], f32)
                    for nd in range(2):
                        nc.tensor.matmul(pf[:, nd * 512:(nd + 1) * 512],
                                         nrm[:, m * 128:(m + 1) * 128],
                                         wo_sb[:, nd * 512:(nd + 1) * 512],
                                         start=True, stop=True)
                    fo = fs.tile([128, DM], f32)
                    nc.vector.tensor_add(fo, pf, be_sb)
                    nc.sync.dma_start(out=d_y[m * 128:(m + 1) * 128, :], in_=fo)

    nc.compile()
    return nc


def _prep_consts(w_in, b_in, w_out, b_out):
    wq = w_in[0:64].astype(np.float64) / 8.0
    wk = w_in[64:128].astype(np.float64)
    wv = w_in[128:192]
    bq = b_in[0:64].astype(np.float64) / 8.0
    bk = b_in[64:128]
    bv = b_in[128:192]

    # strip-layout padded projection weights: head h=4g+s dim d ->
    # column 32s+d of group g
    wq_p = np.zeros((DM, 4, 128), F32)
    wk_p = np.zeros((DM, 4, 128), F32)
    bq_p = np.zeros((128, 4), F32)
    bk_p = np.zeros((128, 4), F32)
    for g in range(4):
        for s in range(4):
            h = 4 * g + s
            for d in range(HD):
                wq_p[:, g, 32 * s + d] = wq[4 * h + d]
                wk_p[:, g, 32 * s + d] = wk[4 * h + d]
                bq_p[32 * s + d, g] = bq[4 * h + d]
                bk_p[32 * s + d, g] = bk[4 * h + d]

    sel = np.zeros((128, NH), F32)
    rep = np.zeros((NH, 128), F32)
    wo = np.zeros((128, DM), F32)
    for h in range(NH):
        base = _slot(h)
        sel[base + 4, h] = 1.0
        for q in range(5):
            rep[h, base + q] = 1.0
        for d in range(HD):
            wo[base + d, :] = w_out[:, 4 * h + d]
    be = (b_out.astype(np.float64) + w_out.astype(np.float64) @ bv.astype(np.float64))

    return {
        "wq": wq_p.astype(BF16), "wk": wk_p.astype(BF16),
        "wv": wv.T.astype(BF16),
        "bq": bq_p.astype(F32), "bk": bk_p.astype(F32),
        "sel": sel, "rep": rep, "wo": wo.astype(BF16),
        "be": be.astype(F32).reshape(1, DM),
    }


def kernel(x, w_in, b_in, w_out, b_out, _trace=False, **kw):
    x = np.asarray(x, F32)
    consts = _prep_consts(np.asarray(w_in, F32), np.asarray(b_in, F32),
                          np.asarray(w_out, F32), np.asarray(b_out, F32))
    if "nc" not in _cache:
        _cache["nc"] = _build_nc()
    nc = _cache["nc"]

    xTs = [np.ascontiguousarray(x[b].T).astype(BF16) for b in range(B)]
    in_maps = []
    for core in range(NC_CORES):
        b, half = divmod(core, 2)
        m = dict(consts)
        m["xT"] = xTs[b]
        m["xqT"] = np.ascontiguousarray(xTs[b][:, half * SQ:(half + 1) * SQ])
        in_maps.append(m)

    res = run_bass_kernel_spmd(nc, in_maps, list(range(NC_CORES)),
                               trace=_trace)
    out = np.empty((B, S, DM), F32)
    for core in range(NC_CORES):
        b, half = divmod(core, 2)
        out[b, half * SQ:(half + 1) * SQ, :] = res.results[core]["y"]
    if _trace:
        return out, res
    return out



# revision 3
# speedup vs baseline: 1.4069x; 1.4069x over previous
"""Trainium2 Bass kernel for nn_MultiHeadedAttention_33835752358170.

Shapes (hardcoded): x [4, 2048, 1024] f32, w_in [192, 1024], b_in [192],
w_out [1024, 64], b_out [1024].  Module quirk: d_k = 64 total across 16
heads -> head_dim = 4.  Scale 1/sqrt(64) = 1/8 folded into q projection.

Sharding: 8 cores = 4 batches x 2 query-halves.  Each core computes
K/V over its batch's full sequence (S=2048) and attention + output
projection for its own 1024 query rows.  The host rotates each core's
xT so its own query half is always columns 0..1023 (softmax over l is
order-invariant, so K/V column order doesn't matter).

Per-core kernel structure (v3):
- scores TRANSPOSED: S^T[l, sq] PSUM tiles [128, 1024] per (head, c),
  K=4 matmuls from strip-layout qT/kT (head h=4g+s at partitions
  [32s,32s+4) of strip g).
- exp split across TWO engines: ScalarE true exp -> bf16, DVE
  Schraudolph fast-exp (x*128/ln2 + 16250 -> int16, bitcast bf16) on
  an interleaved subset of l-chunks.  Worst-case rel-err ~1.2e-2 vs
  the 2e-2 budget (validated in numpy against the f64 reference).
- A@V with E STATIONARY: out[sq,5] += E_chunk[l=128,sq=128]^T @
  V_aug[l=128,5]; ones column accumulates softmax denominators.
  PSUM zero regions are bank-granular, so the 8 sq-chunk accumulation
  groups run m-outer/c-inner (sequential groups in one bank); each
  head's A@V burst is emitted one head late so exp engines never wait
  (32-tile E ring).
- normalize: PE transpose (bf16) of [sq, 64dims+16dens] -> [80, sq],
  DVE reciprocal on partitions 64:80 (lane-aligned), 0/1-matmul
  expands dens 16->64 rows, one DVE multiply -> attnT bf16.
- output projection w/ bias folded as two extra contraction rows
  (be_hi/be_lo); y written bf16 (host upcasts to f32).
"""

import math

import numpy as np
import ml_dtypes

import concourse.bass as bass
import concourse.mybir as mybir
import concourse.tile as tile
from concourse import bacc
from concourse.bass_utils import run_bass_kernel_spmd

BF16 = ml_dtypes.bfloat16
F32 = np.float32

B, S, DM = 4, 2048, 1024
NH, DK = 16, 64
HD = 4          # head dim
SQ = 1024       # query rows per core
NC_CORES = 8

# Schraudolph fast-exp constants (bf16 target: 2^7 mantissa scale)
SCH_A = float(np.float32(128.0 / math.log(2.0)))
SCH_B = float(np.float32(127 * 128 - 6.0))
# l-chunks handled by DVE fast-exp (rest on ScalarE true exp)
DVE_C = frozenset({1, 3, 5, 7, 9, 11, 13})

_cache = {}


def _build_nc():
    f32 = mybir.dt.float32
    bf16 = mybir.dt.bfloat16
    i16 = mybir.dt.int16
    Exp = mybir.ActivationFunctionType.Exp
    mult = mybir.AluOpType.mult
    add = mybir.AluOpType.add

    nc = bacc.Bacc("TRN2", target_bir_lowering=False, debug=False)

    # ---- DRAM I/O ----
    d_xT = nc.dram_tensor("xT", [DM, S], bf16, kind="ExternalInput").ap()
    d_wq = nc.dram_tensor("wq", [DM, 4, 128], bf16, kind="ExternalInput").ap()
    d_wk = nc.dram_tensor("wk", [DM, 4, 128], bf16, kind="ExternalInput").ap()
    d_wv = nc.dram_tensor("wv", [DM, DK], bf16, kind="ExternalInput").ap()
    d_bq = nc.dram_tensor("bq", [128, 4], f32, kind="ExternalInput").ap()
    d_bk = nc.dram_tensor("bk", [128, 4], f32, kind="ExternalInput").ap()
    d_id = nc.dram_tensor("ident", [128, 128], bf16, kind="ExternalInput").ap()
    d_rep = nc.dram_tensor("rep", [80, DK], f32, kind="ExternalInput").ap()
    d_wo = nc.dram_tensor("wo", [66, DM], bf16, kind="ExternalInput").ap()
    d_y = nc.dram_tensor("y", [SQ, DM], bf16, kind="ExternalOutput").ap()

    def big_in(dst, dram_ap, inner):
        # one DMA: DRAM [1024, inner] -> SBUF [128, 8, inner]
        src = bass.AP(tensor=dram_ap.tensor, offset=dram_ap.offset,
                      ap=[[inner, 128], [128 * inner, 8], [1, inner]])
        nc.sync.dma_start(out=dst, in_=src)

    with tile.TileContext(nc) as tc:
        with tc.tile_pool(name="const", bufs=1) as cp:
            xT_sb = cp.tile([128, 8, S], bf16)
            wq_sb = cp.tile([128, 8, 4, 128], bf16)
            wk_sb = cp.tile([128, 8, 4, 128], bf16)
            wv_sb = cp.tile([128, 8, DK], bf16)
            big_in(xT_sb, d_xT, S)
            big_in(wq_sb.rearrange("p c g r -> p c (g r)"), d_wq, 512)
            big_in(wk_sb.rearrange("p c g r -> p c (g r)"), d_wk, 512)
            big_in(wv_sb, d_wv, DK)
            bq_sb = cp.tile([128, 4], f32)
            bk_sb = cp.tile([128, 4], f32)
            id_sb = cp.tile([128, 128], bf16)
            rep_sb = cp.tile([80, DK], f32)
            wo_sb = cp.tile([66, DM], bf16)
            nc.sync.dma_start(out=bq_sb, in_=d_bq)
            nc.sync.dma_start(out=bk_sb, in_=d_bk)
            nc.sync.dma_start(out=id_sb, in_=d_id)
            nc.sync.dma_start(out=rep_sb, in_=d_rep)
            nc.sync.dma_start(out=wo_sb, in_=d_wo)

            qT = cp.tile([128, 4, SQ], bf16)     # strip g: heads 4g..4g+3
            kT = cp.tile([128, 4, S], bf16)
            va = cp.tile([128, 16, NH, 5], bf16)  # (l-chunk, head, 4dims+one)
            araw = cp.tile([128, 8, 80], bf16)   # [sq, m, 64 dims + 16 dens]
            attnT = cp.tile([66, 8, 128], bf16)  # rows 64,65 = ones (bias)
            rcp_sb = cp.tile([80, 8, 128], f32)  # rows 64:80 used

            nc.vector.memset(va, 0.0)
            nc.vector.memset(va[:, :, :, 4:5], 1.0)
            nc.vector.memset(attnT[64:66, :, :], 1.0)

            # ---- projections ----
            with tc.tile_pool(name="pp", bufs=2, space="PSUM") as pp, \
                 tc.tile_pool(name="pv", bufs=2, space="PSUM") as pvp:
                for g in range(4):
                    pt = pp.tile([128, SQ], f32, tag="pt")
                    for nh in range(2):
                        for kc in range(8):
                            nc.tensor.matmul(
                                pt[:, nh * 512:(nh + 1) * 512],
                                wq_sb[:, kc, g, :],
                                xT_sb[:, kc, nh * 512:(nh + 1) * 512],
                                start=(kc == 0), stop=(kc == 7))
                    nc.vector.tensor_scalar_add(qT[:, g, :], pt, bq_sb[:, g:g + 1])
                for g in range(4):
                    for sh in range(2):
                        pt = pp.tile([128, 1024], f32, tag="pt")
                        for nh in range(2):
                            for kc in range(8):
                                nc.tensor.matmul(
                                    pt[:, nh * 512:(nh + 1) * 512],
                                    wk_sb[:, kc, g, :],
                                    xT_sb[:, kc, sh * 1024 + nh * 512: sh * 1024 + (nh + 1) * 512],
                                    start=(kc == 0), stop=(kc == 7))
                        nc.vector.tensor_scalar_add(
                            kT[:, g, sh * 1024:(sh + 1) * 1024], pt, bk_sb[:, g:g + 1])
                for c in range(16):
                    pv = pvp.tile([128, DK], f32, tag="pv")
                    for kc in range(8):
                        nc.tensor.matmul(
                            pv, xT_sb[:, kc, c * 128:(c + 1) * 128],
                            wv_sb[:, kc, :], start=(kc == 0), stop=(kc == 7))
                    pvr = pv.rearrange("p (h d) -> p h d", d=4)
                    nc.vector.tensor_copy(va[:, c, :, 0:4], pvr)

            # ---- attention main loop (per head; A@V delayed one head) ----
            with tc.tile_pool(name="op", bufs=2, space="PSUM") as op, \
                 tc.tile_pool(name="sp", bufs=3, space="PSUM") as sp, \
                 tc.tile_pool(name="ep", bufs=32) as ep:
                pending = None  # (head, av_tile, [16 et bf16 views])

                def av_burst(ph, pav, pets):
                    for m in range(8):
                        for c in range(16):
                            nc.tensor.matmul(
                                pav[:, m, 0:5],
                                pets[c][:, m * 128:(m + 1) * 128],
                                va[:, c, ph, :],
                                start=(c == 0), stop=(c == 15))
                    nc.vector.tensor_copy(araw[:, :, 4 * ph:4 * ph + 4],
                                          pav[:, :, 0:4])
                    nc.vector.tensor_copy(araw[:, :, 64 + ph:65 + ph],
                                          pav[:, :, 4:5])

                for h in range(NH):
                    g, s = divmod(h, 4)
                    av = op.tile([128, 8, 8], f32, tag="av")
                    ets = []
                    for c in range(16):
                        st = sp.tile([128, 1024], f32, tag="st")
                        for nh in range(2):
                            nc.tensor.matmul(
                                st[:, nh * 512:(nh + 1) * 512],
                                kT[32 * s:32 * s + 4, g, c * 128:(c + 1) * 128],
                                qT[32 * s:32 * s + 4, g, nh * 512:(nh + 1) * 512],
                                start=True, stop=True,
                                tile_position=(32 * s, 0))
                        et = ep.tile([128, 1024], i16, tag="et")
                        etb = et.bitcast(bf16)
                        if c in DVE_C:
                            nc.vector.tensor_scalar(et, st, SCH_A, SCH_B, mult, add)
                        else:
                            nc.scalar.activation(etb, st, Exp)
                        ets.append(etb)
                    if pending is not None:
                        av_burst(*pending)
                    pending = (h, av, ets)
                av_burst(*pending)

            # ---- normalize + output projection ----
            with tc.tile_pool(name="tpp", bufs=2, space="PSUM") as tpp, \
                 tc.tile_pool(name="rpp", bufs=2, space="PSUM") as rpp, \
                 tc.tile_pool(name="ypp", bufs=2, space="PSUM") as ypp, \
                 tc.tile_pool(name="fs", bufs=2) as fs:
                for m in range(8):
                    tp = tpp.tile([80, 128], bf16, tag="tp")
                    nc.tensor.transpose(tp, araw[:, m, :], id_sb)
                    nc.vector.reciprocal(rcp_sb[64:80, m, :], tp[64:80, :])
                    rp = rpp.tile([DK, 128], f32, tag="rp")
                    nc.tensor.matmul(rp, rep_sb[64:80, :], rcp_sb[64:80, m, :],
                                     start=True, stop=True,
                                     tile_position=(64, 0))
                    rps = fs.tile([DK, 128], f32, tag="rps")
                    nc.vector.tensor_copy(rps, rp)
                    nc.vector.tensor_mul(attnT[0:64, m, :], tp[0:64, :], rps)
                    yp = ypp.tile([128, DM], f32, tag="yp")
                    for nd in range(2):
                        nc.tensor.matmul(yp[:, nd * 512:(nd + 1) * 512],
                                         attnT[:, m, :],
                                         wo_sb[:, nd * 512:(nd + 1) * 512],
                                         start=True, stop=True)
                    ys = fs.tile([128, DM], bf16, tag="ys")
                    if m % 2 == 0:
                        nc.scalar.copy(ys, yp)
                    else:
                        nc.vector.tensor_copy(ys, yp)
                    nc.sync.dma_start(out=d_y[m * 128:(m + 1) * 128, :], in_=ys)

    nc.compile()
    return nc


def _prep_consts(w_in, b_in, w_out, b_out):
    wq = w_in[0:64].astype(np.float64) / 8.0
    wk = w_in[64:128].astype(np.float64)
    wv = w_in[128:192]
    bq = b_in[0:64].astype(np.float64) / 8.0
    bk = b_in[64:128]
    bv = b_in[128:192]

    # strip-layout padded projection weights: head h=4g+s dim d ->
    # column 32s+d of group g
    wq_p = np.zeros((DM, 4, 128), F32)
    wk_p = np.zeros((DM, 4, 128), F32)
    bq_p = np.zeros((128, 4), F32)
    bk_p = np.zeros((128, 4), F32)
    for g in range(4):
        for s in range(4):
            h = 4 * g + s
            for d in range(HD):
                wq_p[:, g, 32 * s + d] = wq[4 * h + d]
                wk_p[:, g, 32 * s + d] = wk[4 * h + d]
                bq_p[32 * s + d, g] = bq[4 * h + d]
                bk_p[32 * s + d, g] = bk[4 * h + d]

    rep = np.zeros((80, DK), F32)   # rows 64:80 = head->dims expand
    for h in range(NH):
        for d in range(HD):
            rep[64 + h, 4 * h + d] = 1.0

    be = (b_out.astype(np.float64) + w_out.astype(np.float64) @ bv.astype(np.float64)).astype(F32)
    be_hi = be.astype(BF16).astype(F32)
    be_lo = be - be_hi
    wo_aug = np.zeros((66, DM), F32)
    wo_aug[0:64, :] = w_out.T
    wo_aug[64, :] = be_hi
    wo_aug[65, :] = be_lo

    return {
        "wq": wq_p.astype(BF16), "wk": wk_p.astype(BF16),
        "wv": wv.T.astype(BF16),
        "bq": bq_p.astype(F32), "bk": bk_p.astype(F32),
        "ident": np.eye(128, dtype=F32).astype(BF16),
        "rep": rep,
        "wo": wo_aug.astype(BF16),
    }


def kernel(x, w_in, b_in, w_out, b_out, _trace=False, **kw):
    x = np.asarray(x, F32)
    consts = _prep_consts(np.asarray(w_in, F32), np.asarray(b_in, F32),
                          np.asarray(w_out, F32), np.asarray(b_out, F32))
    if "nc" not in _cache:
        _cache["nc"] = _build_nc()
    nc = _cache["nc"]

    xTs = [np.ascontiguousarray(x[b].T).astype(BF16) for b in range(B)]
    in_maps = []
    for core in range(NC_CORES):
        b, half = divmod(core, 2)
        m = dict(consts)
        if half == 0:
            m["xT"] = xTs[b]
        else:
            # rotate so this core's query half is columns 0..1023
            m["xT"] = np.ascontiguousarray(
                np.concatenate([xTs[b][:, SQ:], xTs[b][:, :SQ]], axis=1))
        in_maps.append(m)

    res = run_bass_kernel_spmd(nc, in_maps, list(range(NC_CORES)),
                               trace=_trace)
    out = np.empty((B, S, DM), F32)
    for core in range(NC_CORES):
        b, half = divmod(core, 2)
        out[b, half * SQ:(half + 1) * SQ, :] = res.results[core]["y"].astype(F32)
    if _trace:
        return out, res
    return out


# revision 5
# speedup vs baseline: 1.5102x; 1.0734x over previous
"""Trainium2 Bass kernel for nn_MultiHeadedAttention_33835752358170.

Shapes (hardcoded): x [4, 2048, 1024] f32, w_in [192, 1024], b_in [192],
w_out [1024, 64], b_out [1024].  Module quirk: d_k = 64 total across 16
heads -> head_dim = 4.  Scale 1/sqrt(64) = 1/8 folded into q projection.

Sharding: 8 cores = 4 batches x 2 query-halves.  Each core computes
K/V over its batch's full sequence (S=2048) and attention + output
projection for its own 1024 query rows.  The host rotates each core's
xT so its own query half is always columns 0..1023 (softmax over l is
order-invariant, so K/V column order doesn't matter).

Per-core kernel structure (v4):
- scores TRANSPOSED: S^T[l, sq] PSUM tiles [128, 1024] per (head, c),
  K=4 matmuls from strip-layout qT/kT (head h=4g+s at partitions
  [32s,32s+4) of strip g).
- exp split across TWO engines: ScalarE true exp -> bf16, DVE
  Schraudolph fast-exp (x*128/ln2 + 16250 -> int16, bitcast bf16) on
  an interleaved subset of l-chunks.  Worst-case rel-err ~1.2e-2 vs
  the 2e-2 budget (validated in numpy against the f64 reference).
- A@V with E STATIONARY: out[sq,5] += E_chunk[l=128,sq=128]^T @
  V_aug[l=128,5]; ones column accumulates softmax denominators.
  PSUM zero regions are bank-granular, so the 8 sq-chunk accumulation
  groups run m-outer/c-inner (sequential groups in one bank); each
  head's A@V burst is emitted one head late so exp engines never wait
  (32-tile E ring).
- warm start: only strip-0 q/k projections + V run before the head
  loop; strips 1-3 are interleaved into heads 0..11 through the score
  PSUM ring (PE has slack there), so ScalarE starts exp'ing ~20us in
  instead of ~77us.
- normalize: PE transpose (bf16) of [sq, 64dims+16dens] -> [80, sq],
  DVE reciprocal on partitions 64:80 (lane-aligned), 0/1-matmul
  expands dens 16->64 rows, one DVE multiply -> attnT bf16.
- output projection w/ bias folded as two extra contraction rows
  (be_hi/be_lo); y written bf16 (host upcasts to f32).
"""

import math

import numpy as np
import ml_dtypes

import concourse.bass as bass
import concourse.mybir as mybir
import concourse.tile as tile
from concourse import bacc
from concourse.bass_utils import run_bass_kernel_spmd

BF16 = ml_dtypes.bfloat16
F32 = np.float32

B, S, DM = 4, 2048, 1024
NH, DK = 16, 64
HD = 4          # head dim
SQ = 1024       # query rows per core
NC_CORES = 8

# Schraudolph fast-exp constants (bf16 target: 2^7 mantissa scale)
SCH_A = float(np.float32(128.0 / math.log(2.0)))
SCH_B = float(np.float32(127 * 128 - 6.0))
# l-chunks handled by DVE fast-exp (rest on ScalarE true exp)
DVE_C7 = frozenset({1, 3, 5, 7, 9, 11, 13})
DVE_C6 = frozenset({1, 3, 5, 7, 9, 11})


def _dve_set(h):
    return DVE_C6 if h in (7, 15) else DVE_C7


_cache = {}


def _build_nc():
    f32 = mybir.dt.float32
    bf16 = mybir.dt.bfloat16
    i16 = mybir.dt.int16
    Exp = mybir.ActivationFunctionType.Exp
    mult = mybir.AluOpType.mult
    add = mybir.AluOpType.add

    nc = bacc.Bacc("TRN2", target_bir_lowering=False, debug=False)

    # ---- DRAM I/O ----
    d_xT = nc.dram_tensor("xT", [DM, S], bf16, kind="ExternalInput").ap()
    d_wq = nc.dram_tensor("wq", [DM, 4, 128], bf16, kind="ExternalInput").ap()
    d_wk = nc.dram_tensor("wk", [DM, 4, 128], bf16, kind="ExternalInput").ap()
    d_wv = nc.dram_tensor("wv", [DM, DK], bf16, kind="ExternalInput").ap()
    d_bq = nc.dram_tensor("bq", [128, 4], f32, kind="ExternalInput").ap()
    d_bk = nc.dram_tensor("bk", [128, 4], f32, kind="ExternalInput").ap()
    d_id = nc.dram_tensor("ident", [128, 128], bf16, kind="ExternalInput").ap()
    d_rep = nc.dram_tensor("rep", [80, DK], f32, kind="ExternalInput").ap()
    d_wo = nc.dram_tensor("wo", [66, DM], bf16, kind="ExternalInput").ap()
    d_y = nc.dram_tensor("y", [SQ, DM], bf16, kind="ExternalOutput").ap()

    def slab(dram_ap, inner, col0, ncol, kc0, nkc):
        # DRAM [1024, inner] -> SBUF [128, nkc, ncol] (kc-chunked rows)
        return bass.AP(tensor=dram_ap.tensor,
                       offset=dram_ap.offset + kc0 * 128 * inner + col0,
                       ap=[[inner, 128], [128 * inner, nkc], [1, ncol]])

    with tile.TileContext(nc) as tc:
        with tc.tile_pool(name="const", bufs=1) as cp:
            xT_sb = cp.tile([128, 8, S], bf16)
            wq_sb = cp.tile([128, 8, 4, 128], bf16)
            wk_sb = cp.tile([128, 8, 4, 128], bf16)
            wv_sb = cp.tile([128, 8, DK], bf16)
            bq_sb = cp.tile([128, 4], f32)
            bk_sb = cp.tile([128, 4], f32)
            id_sb = cp.tile([128, 128], bf16)
            rep_sb = cp.tile([80, DK], f32)
            wo_sb = cp.tile([66, DM], bf16)
            # ordered so q-strip-0 prerequisites land first
            nc.sync.dma_start(out=bq_sb, in_=d_bq)
            nc.sync.dma_start(out=bk_sb, in_=d_bk)
            nc.sync.dma_start(out=wq_sb.rearrange("p c g r -> p c (g r)"),
                              in_=slab(d_wq, 512, 0, 512, 0, 8))
            nc.sync.dma_start(out=xT_sb[:, 0:4, 0:SQ],
                              in_=slab(d_xT, S, 0, SQ, 0, 4))
            nc.sync.dma_start(out=xT_sb[:, 4:8, 0:SQ],
                              in_=slab(d_xT, S, 0, SQ, 4, 4))
            nc.sync.dma_start(out=wk_sb.rearrange("p c g r -> p c (g r)"),
                              in_=slab(d_wk, 512, 0, 512, 0, 8))
            nc.sync.dma_start(out=xT_sb[:, 0:4, SQ:S],
                              in_=slab(d_xT, S, SQ, SQ, 0, 4))
            nc.sync.dma_start(out=xT_sb[:, 4:8, SQ:S],
                              in_=slab(d_xT, S, SQ, SQ, 4, 4))
            nc.sync.dma_start(out=wv_sb, in_=slab(d_wv, DK, 0, DK, 0, 8))
            nc.sync.dma_start(out=id_sb, in_=d_id)
            nc.sync.dma_start(out=rep_sb, in_=d_rep)
            nc.sync.dma_start(out=wo_sb, in_=d_wo)

            qT = cp.tile([128, 4, SQ], bf16)     # strip g: heads 4g..4g+3
            kT = cp.tile([128, 4, S], bf16)
            va = cp.tile([128, 16, NH, 5], bf16)  # (l-chunk, head, 4dims+one)
            araw = cp.tile([128, 8, 80], bf16)   # [sq, m, 64 dims + 16 dens]
            attnT = cp.tile([66, 8, 128], bf16)  # rows 64,65 = ones (bias)
            rcp_sb = cp.tile([80, 8, 128], f32)  # rows 64:80 used

            nc.gpsimd.memset(va, 0.0)
            nc.gpsimd.memset(va[:, :, :, 4:5], 1.0)
            nc.gpsimd.memset(attnT[64:66, :, :], 1.0)

            def q_mms(pool, g, tag="st"):
                cell = {}

                def get_pt():
                    if "pt" not in cell:
                        cell["pt"] = pool.tile([128, SQ], f32, tag=tag,
                                               name=f"ptq{g}")
                    return cell["pt"]

                pieces = []
                for nh in range(2):
                    for kc0 in (0, 4):
                        def mk(nh=nh, kc0=kc0):
                            pt = get_pt()
                            for kc in range(kc0, kc0 + 4):
                                nc.tensor.matmul(
                                    pt[:, nh * 512:(nh + 1) * 512],
                                    wq_sb[:, kc, g, :],
                                    xT_sb[:, kc, nh * 512:(nh + 1) * 512],
                                    start=(kc == 0), stop=(kc == 7))
                        pieces.append(mk)

                def bias(g=g):
                    nc.scalar.add(qT[:, g, :], get_pt(), bq_sb[:, g:g + 1])
                pieces.append(bias)
                return pieces

            def k_mms(pool, g, sh, tag="st"):
                cell = {}

                def get_pt():
                    if "pt" not in cell:
                        cell["pt"] = pool.tile([128, 1024], f32, tag=tag,
                                               name=f"ptk{g}{sh}")
                    return cell["pt"]

                pieces = []
                for nh in range(2):
                    for kc0 in (0, 4):
                        def mk(nh=nh, kc0=kc0):
                            pt = get_pt()
                            for kc in range(kc0, kc0 + 4):
                                nc.tensor.matmul(
                                    pt[:, nh * 512:(nh + 1) * 512],
                                    wk_sb[:, kc, g, :],
                                    xT_sb[:, kc, sh * 1024 + nh * 512:
                                          sh * 1024 + (nh + 1) * 512],
                                    start=(kc == 0), stop=(kc == 7))
                        pieces.append(mk)

                def bias(g=g, sh=sh):
                    nc.vector.tensor_scalar_add(
                        kT[:, g, sh * 1024:(sh + 1) * 1024], get_pt(),
                        bk_sb[:, g:g + 1])
                pieces.append(bias)
                return pieces

            # ---- phase A: strip-0 projections + V ----
            with tc.tile_pool(name="pp", bufs=2, space="PSUM") as pp, \
                 tc.tile_pool(name="pv", bufs=2, space="PSUM") as pvp:
                for p in q_mms(pp, 0, tag="pt"):
                    p()
                for sh in range(2):
                    for p in k_mms(pp, 0, sh, tag="pt"):
                        p()
                for c in range(16):
                    pv = pvp.tile([128, DK], f32, tag="pv")
                    for kc in range(8):
                        nc.tensor.matmul(
                            pv, xT_sb[:, kc, c * 128:(c + 1) * 128],
                            wv_sb[:, kc, :], start=(kc == 0), stop=(kc == 7))
                    pvr = pv.rearrange("p (h d) -> p h d", d=4)
                    nc.vector.tensor_copy(va[:, c, :, 0:4], pvr)

            # ---- phase B: attention (strips 1-3 interleaved via sp ring) ----
            with tc.tile_pool(name="op", bufs=2, space="PSUM") as op, \
                 tc.tile_pool(name="sp", bufs=3, space="PSUM") as sp, \
                 tc.tile_pool(name="ep", bufs=32) as ep:
                pending = None  # (head, av_tile, [16 et bf16 views])

                def av_burst(ph, pav, pets):
                    for m in range(8):
                        for c in range(16):
                            nc.tensor.matmul(
                                pav[:, m, 0:5],
                                pets[c][:, m * 128:(m + 1) * 128],
                                va[:, c, ph, :],
                                start=(c == 0), stop=(c == 15))
                    nc.vector.tensor_copy(araw[:, :, 4 * ph:4 * ph + 4],
                                          pav[:, :, 0:4])
                    nc.vector.tensor_copy(araw[:, :, 64 + ph:65 + ph],
                                          pav[:, :, 4:5])

                # deferred projection pieces: strip g emitted during heads
                # 4(g-1)..4g-1, three pieces per head
                deferred = {}
                for g in (1, 2, 3):
                    deferred[g] = (q_mms(sp, g) + k_mms(sp, g, 0)
                                   + k_mms(sp, g, 1))

                for h in range(NH):
                    g, s = divmod(h, 4)
                    av = op.tile([128, 8, 8], f32, tag="av")
                    ets = []
                    dset = _dve_set(h)
                    pieces = deferred.get(h // 4 + 1, []) if h < 12 else []
                    r = h % 4
                    my_pieces = pieces[4 * r:4 * r + 4]
                    for c in range(16):
                        st = sp.tile([128, 1024], f32, tag="st")
                        for nh in range(2):
                            nc.tensor.matmul(
                                st[:, nh * 512:(nh + 1) * 512],
                                kT[32 * s:32 * s + 4, g, c * 128:(c + 1) * 128],
                                qT[32 * s:32 * s + 4, g, nh * 512:(nh + 1) * 512],
                                start=True, stop=True,
                                tile_position=(32 * s, 0))
                        et = ep.tile([128, 1024], i16, tag="et")
                        etb = et.bitcast(bf16)
                        if c in dset:
                            nc.vector.tensor_scalar(et, st, SCH_A, SCH_B, mult, add)
                        else:
                            nc.scalar.activation(etb, st, Exp)
                        ets.append(etb)
                        if c in (3, 7, 11, 15) and my_pieces:
                            my_pieces.pop(0)()
                    if pending is not None:
                        av_burst(*pending)
                    pending = (h, av, ets)
                av_burst(*pending)

            # ---- normalize + output projection ----
            with tc.tile_pool(name="tpp", bufs=2, space="PSUM") as tpp, \
                 tc.tile_pool(name="rpp", bufs=2, space="PSUM") as rpp, \
                 tc.tile_pool(name="ypp", bufs=4, space="PSUM") as ypp, \
                 tc.tile_pool(name="fs", bufs=2) as fs:
                for m in range(8):
                    tp = tpp.tile([80, 128], bf16, tag="tp")
                    nc.tensor.transpose(tp, araw[:, m, :], id_sb)
                    nc.vector.reciprocal(rcp_sb[64:80, m, :], tp[64:80, :])
                    rp = rpp.tile([DK, 128], f32, tag="rp")
                    nc.tensor.matmul(rp, rep_sb[64:80, :], rcp_sb[64:80, m, :],
                                     start=True, stop=True,
                                     tile_position=(64, 0))
                    rps = fs.tile([DK, 128], f32, tag="rps")
                    nc.vector.tensor_copy(rps, rp)
                    nc.vector.tensor_mul(attnT[0:64, m, :], tp[0:64, :], rps)
                    ys = fs.tile([128, DM], bf16, tag="ys")
                    for nd in range(2):
                        yp = ypp.tile([128, 512], f32, tag="yp")
                        nc.tensor.matmul(yp, attnT[:, m, :],
                                         wo_sb[:, nd * 512:(nd + 1) * 512],
                                         start=True, stop=True)
                        if m % 2 == 0:
                            nc.scalar.copy(ys[:, nd * 512:(nd + 1) * 512], yp)
                        else:
                            nc.vector.tensor_copy(ys[:, nd * 512:(nd + 1) * 512], yp)
                    nc.sync.dma_start(out=d_y[m * 128:(m + 1) * 128, :], in_=ys)

    nc.compile()
    return nc


def _prep_consts(w_in, b_in, w_out, b_out):
    wq = w_in[0:64].astype(np.float64) / 8.0
    wk = w_in[64:128].astype(np.float64)
    wv = w_in[128:192]
    bq = b_in[0:64].astype(np.float64) / 8.0
    bk = b_in[64:128]
    bv = b_in[128:192]

    # strip-layout padded projection weights: head h=4g+s dim d ->
    # column 32s+d of group g
    wq_p = np.zeros((DM, 4, 128), F32)
    wk_p = np.zeros((DM, 4, 128), F32)
    bq_p = np.zeros((128, 4), F32)
    bk_p = np.zeros((128, 4), F32)
    for g in range(4):
        for s in range(4):
            h = 4 * g + s
            for d in range(HD):
                wq_p[:, g, 32 * s + d] = wq[4 * h + d]
                wk_p[:, g, 32 * s + d] = wk[4 * h + d]
                bq_p[32 * s + d, g] = bq[4 * h + d]
                bk_p[32 * s + d, g] = bk[4 * h + d]

    rep = np.zeros((80, DK), F32)   # rows 64:80 = head->dims expand
    for h in range(NH):
        for d in range(HD):
            rep[64 + h, 4 * h + d] = 1.0

    be = (b_out.astype(np.float64) + w_out.astype(np.float64) @ bv.astype(np.float64)).astype(F32)
    be_hi = be.astype(BF16).astype(F32)
    be_lo = be - be_hi
    wo_aug = np.zeros((66, DM), F32)
    wo_aug[0:64, :] = w_out.T
    wo_aug[64, :] = be_hi
    wo_aug[65, :] = be_lo

    return {
        "wq": wq_p.astype(BF16), "wk": wk_p.astype(BF16),
        "wv": wv.T.astype(BF16),
        "bq": bq_p.astype(F32), "bk": bk_p.astype(F32),
        "ident": np.eye(128, dtype=F32).astype(BF16),
        "rep": rep,
        "wo": wo_aug.astype(BF16),
    }


def kernel(x, w_in, b_in, w_out, b_out, _trace=False, **kw):
    x = np.asarray(x, F32)
    consts = _prep_consts(np.asarray(w_in, F32), np.asarray(b_in, F32),
                          np.asarray(w_out, F32), np.asarray(b_out, F32))
    if "nc" not in _cache:
        _cache["nc"] = _build_nc()
    nc = _cache["nc"]

    xTs = [np.ascontiguousarray(x[b].T).astype(BF16) for b in range(B)]
    in_maps = []
    for core in range(NC_CORES):
        b, half = divmod(core, 2)
        m = dict(consts)
        if half == 0:
            m["xT"] = xTs[b]
        else:
            # rotate so this core's query half is columns 0..1023
            m["xT"] = np.ascontiguousarray(
                np.concatenate([xTs[b][:, SQ:], xTs[b][:, :SQ]], axis=1))
        in_maps.append(m)

    res = run_bass_kernel_spmd(nc, in_maps, list(range(NC_CORES)),
                               trace=_trace)
    out = np.empty((B, S, DM), F32)
    for core in range(NC_CORES):
        b, half = divmod(core, 2)
        out[b, half * SQ:(half + 1) * SQ, :] = res.results[core]["y"].astype(F32)
    if _trace:
        return out, res
    return out
